# revision 51
# baseline (speedup 1.0000x reference)
"""Single-head attention on 8 Trainium2 NeuronCores, batch-sharded.

Per core (one batch element b). Host-side layouts make every DMA a large
contiguous read (3KB/partition halves of x chunks).

Projections (bf16, chunk order 0..3, two fused groups):
  A [Wv|Wq] -> vq tile: rows 0-63 v^T, rows 64-127 q^T  (one DVE add)
  B [Wk]    -> kT tile rows 64-127 (M=64 matmul, tile_position (0,64));
               kT rows 0-63 are memset to zero once.
Scores then run the K=128 contraction directly on these tiles: the zero
rows of kT annihilate the v rows of vq, so NO zero-padded copies of q are
needed anywhere:  scores^T[kt] = kT[:,kt-cols]^T @ vq[:,q-cols].

Scores: 16 serial matmuls [K=128, M=128, N=512] per q-chunk at the warm
issue rate (~216ns); measured row-tiling of K=64 pairs gives NO speedup
(concurrent row-tiles serialize on the moving-operand SBUF port), so the
simple padded form wins (it needs 6144 fewer projection rows).
q-chunk-0 scores for k-tiles 4c..4c+3 are emitted right after chunk c's
projections, so the exp stream starts ~3us into the projection phase.

exp: split across TWO engines writing bf16 P^T tiles from [128,1536]
psum score tiles. ACT handles most groups (exact exp, scale=1/8 folded
in). DVE handles groups 0,3 of q-chunks 1-3 with a Schraudolph bit-trick:
bf16 bits of 2^y are linear in y, so
    bits = round(s_raw * (log2e/8 * 128) + 16248.5)
computed by ONE tensor_scalar (mult+add, f32 psum in, int16 out) IS
exp(s/8) to within ~2%; the int16 tile is bitcast to bf16 for the PV
matmul. This removes exp as the serial bottleneck (sim rel err of the
mix: 0.7e-2 < 2e-2 budget).

PV (bf16): per k-tile matmul, M=65 (V plus a ones row -> softmax
denominator row), accumulated over 16 k-tiles into a 1-bank psum.
PV(qc-1) interleaves with scores(qc); PV(3) trails its own exps by two
groups inside qc=3 to shorten the tail. outq psum banks alternate
between the "o" and "pj" tags (which projections also rotate through
during the projection phase).

V layout: per-chunk PE transposes of vq rows into v65 [128, kt, 80]
(pitch 160B) + DVE copy; a DMA-transpose variant exists but measured
slower (queue issue cost) — kept behind USE_DMA_TRANSPOSE.

Epilogue per q-chunk: DVE copy psum->SBUF, DMA out^T [65,512] f32; the
host does the divide-by-denominator and the final transpose (cheap).

PSUM: tag "sc" 2x3 banks (score tiles), tags "pj"+"o" 1 bank each
(projection psums, V-transpose staging, outq accumulators) = 8 banks.
"""

import numpy as np

USE_DMA_TRANSPOSE = False
VPITCH = 80  # v65 per-k-tile pitch: 160B — every multiple is 32B-aligned
             # (DMA-transpose dest requires 32B alignment)

B, S, D, H = 8, 2048, 768, 64
DT = D // 128          # 6 d-tiles
NQ = S // 512          # 4 q-chunks of 512
NK = S // 128          # 16 k-tiles of 128
SCALE = 1.0 / np.sqrt(H).item()
SCH_A = SCALE * np.log2(np.e).item() * 128.0   # Schraudolph slope
SCH_C = 16248.5                                 # Schraudolph offset (tuned)
GROUPS = ((0, 3), (3, 6), (6, 9), (9, 12), (12, 15), (15, 16))
DVE_GROUPS = {(qc, g) for qc in (1, 2, 3) for g in (0, 3)}

_cache = {}


def _build():
    import concourse.mybir as mybir
    import concourse.tile as tile
    from concourse import bacc
    from concourse.masks import make_identity

    f32 = mybir.dt.float32
    bf16 = mybir.dt.bfloat16
    i16 = mybir.dt.int16
    Exp = mybir.ActivationFunctionType.Exp
    Mult = mybir.AluOpType.mult
    Add = mybir.AluOpType.add

    nc = bacc.Bacc(None)
    xp_d = nc.dram_tensor("xp", [128, NQ, 2, DT * 256], bf16, kind="ExternalInput")
    wA_d = nc.dram_tensor("wA", [128, DT * 128], bf16, kind="ExternalInput")
    wB_d = nc.dram_tensor("wB", [128, DT * 64], bf16, kind="ExternalInput")
    bA_d = nc.dram_tensor("bA", [128, 1], f32, kind="ExternalInput")
    bB_d = nc.dram_tensor("bB", [128, 1], f32, kind="ExternalInput")
    out_d = nc.dram_tensor("out", [H + 1, NQ * 512], f32, kind="ExternalOutput")

    with tile.TileContext(nc) as tc:
        with (
            tc.tile_pool(name="big", bufs=1) as big,
            tc.tile_pool(name="small", bufs=1) as small,
            tc.tile_pool(name="pt", bufs=10) as ptp,
            tc.tile_pool(name="res", bufs=2) as resp,
            tc.tile_pool(name="ps", bufs=2, space="PSUM") as ps,
        ):
            if not USE_DMA_TRANSPOSE:
                ident = small.tile([128, 128], f32)
                make_identity(nc, ident)
                identb = small.tile([128, 128], bf16)
                nc.gpsimd.tensor_copy(out=identb, in_=ident)

            # warm the ACT exp table during DMA fill
            zwarm = small.tile([128, 8], f32)
            nc.gpsimd.memset(zwarm, 0.0)
            wwarm = small.tile([128, 8], bf16)
            nc.scalar.activation(out=wwarm, in_=zwarm, func=Exp)

            # ---- DMAs, ordered so chunk-0 compute starts earliest ----
            wA = small.tile([128, DT, 128], bf16)
            wB = small.tile([128, DT, H], bf16)
            bA = small.tile([128, 1], f32)
            bB = small.tile([128, 1], f32)
            xT = big.tile([128, NQ, 2, DT * 256], bf16)

            nc.sync.dma_start(out=wA, in_=wA_d[:, :].rearrange("p (t h) -> p t h", t=DT))
            nc.sync.dma_start(out=wB, in_=wB_d[:, :].rearrange("p (t h) -> p t h", t=DT))
            nc.sync.dma_start(out=bA, in_=bA_d[:, :])
            nc.sync.dma_start(out=bB, in_=bB_d[:, :])
            for c in range(4):
                for hh in (0, 1):
                    nc.sync.dma_start(out=xT[:, c, hh, :], in_=xp_d[:, c, hh, :])

            # ---- PE warmup: dummy matmuls during the DMA fill get the HAM
            # clock gate to K=8/8 (2.4 GHz) before the first real matmul ----
            wsrc = small.tile([128, 512], bf16)
            nc.gpsimd.memset(wsrc, 1.0)
            wps = ps.tile([128, 512], f32, tag="pj", name="warmps", bufs=1)
            for i in range(20):
                nc.tensor.matmul(wps, lhsT=identb, rhs=wsrc, start=True, stop=True)

            # ---- persistent tensors ----
            vq = big.tile([128, S], bf16, tag="vq")   # v^T lo / q^T hi
            kT = big.tile([128, S], bf16, tag="kT")   # zeros lo / k^T hi
            nc.gpsimd.memset(kT[:H, :], 0.0)
            v65 = big.tile([128, NK, VPITCH], bf16, tag="v65")
            nc.gpsimd.memset(v65[:, :, H : H + 1], 1.0)

            # proj psums / vtrans staging / outq alternate two 1-bank tags
            alt = {"n": 0}

            def pj_tile(cols, nm, dtype=f32):
                tag = ("pj", "o")[alt["n"] % 2]
                alt["n"] += 1
                return ps.tile([128, cols], dtype, tag=tag, name=nm, bufs=1)

            pt_tiles = {}
            sc_state = {}
            outqs = [None] * NQ

            def emit_pv(qc, n):
                g = n // 3
                lo, _hi = GROUPS[g]
                pt = pt_tiles[(qc, g)]
                rhs_t = pt if pt.dtype == bf16 else pt.bitcast(bf16)
                slot = n - lo
                nc.tensor.matmul(
                    outqs[qc],
                    lhsT=v65[:, n, : H + 1],
                    rhs=rhs_t[:, slot * 512 : (slot + 1) * 512],
                    start=(n == 0),
                    stop=(n == NK - 1),
                )

            def emit_epilogue(qc):
                oT = resp.tile([H + 1, 512], f32, tag="oT", name=f"oT{qc}")
                nc.vector.tensor_copy(out=oT, in_=outqs[qc])
                nc.sync.dma_start(
                    out=out_d[:, qc * 512 : (qc + 1) * 512], in_=oT
                )

            def emit_score(qc, n):
                """scores^T for k-tile n against q-chunk qc (K=128 padded)."""
                st = sc_state.setdefault(qc, [None] * len(GROUPS))
                g = n // 3
                lo, hi = GROUPS[g]
                if st[g] is None:
                    st[g] = ps.tile(
                        [128, (hi - lo) * 512], f32, tag="sc", name=f"sc{qc}_{g}"
                    )
                slot = n - lo
                nc.tensor.matmul(
                    st[g][:, slot * 512 : (slot + 1) * 512],
                    lhsT=kT[:, n * 128 : (n + 1) * 128],
                    rhs=vq[:, qc * 512 : (qc + 1) * 512],
                    start=True,
                    stop=True,
                )
                if n == hi - 1:  # group full -> exp
                    cols = (hi - lo) * 512
                    if (qc, g) in DVE_GROUPS:
                        pt = ptp.tile([128, cols], i16, tag="pT", name=f"pt{qc}_{g}")
                        nc.vector.tensor_scalar(
                            out=pt,
                            in0=st[g],
                            scalar1=SCH_A,
                            scalar2=SCH_C,
                            op0=Mult,
                            op1=Add,
                        )
                    else:
                        pt = ptp.tile([128, cols], bf16, tag="pT", name=f"pt{qc}_{g}")
                        nc.scalar.activation(out=pt, in_=st[g], func=Exp, scale=SCALE)
                    pt_tiles[(qc, g)] = pt

            # ---- projection phase ----
            # Dependent work for chunk c (V transposes, q-chunk-0 scores)
            # is emitted AFTER chunk c+1's projection matmuls, so the DVE
            # bias-adds have a full matmul group's time to complete.
            def chunk_tail(c):
                for j in range(4):
                    kt = 4 * c + j
                    if USE_DMA_TRANSPOSE:
                        nc.sync.dma_start(
                            out=v65[:, kt, :H],
                            in_=vq[:H, kt * 128 : (kt + 1) * 128],
                            transpose=True,
                        )
                    else:
                        tp = pj_tile(128, f"vtr{kt}", dtype=bf16)
                        nc.tensor.transpose(
                            tp[:, :H],
                            vq[:H, kt * 128 : (kt + 1) * 128],
                            identb[:H, :H],
                        )
                        nc.vector.tensor_copy(out=v65[:, kt, :H], in_=tp[:, :H])
                for j in range(4):
                    emit_score(0, 4 * c + j)

            for c in range(4):
                cc = slice(c * 512, (c + 1) * 512)

                def proj(w, nm, mlo, mhi, c0=c):
                    p = pj_tile(512, f"ps{nm}{c0}")
                    for hh in (0, 1):
                        for dt in range(DT):
                            nc.tensor.matmul(
                                p[mlo:mhi, hh * 256 : (hh + 1) * 256],
                                lhsT=w[:, dt, :],
                                rhs=xT[:, c0, hh, dt * 256 : (dt + 1) * 256],
                                start=(dt == 0),
                                stop=(dt == DT - 1),
                            )
                    return p

                psA = proj(wA, "A", 0, 128)
                nc.vector.tensor_scalar_add(out=vq[:, cc], in0=psA, scalar1=bA)
                psB = proj(wB, "B", H, 128)
                nc.vector.tensor_scalar_add(
                    out=kT[H:, cc], in0=psB[H:, :], scalar1=bB[H:, :]
                )
                if c >= 1:
                    chunk_tail(c - 1)
            chunk_tail(3)

            # ---- steady phase ----
            pv_cursor = [0] * NQ

            def pump_pv(qc, limit_n):
                while pv_cursor[qc] < min(limit_n, NK):
                    emit_pv(qc, pv_cursor[qc])
                    pv_cursor[qc] += 1

            outqs[0] = ps.tile([H + 1, 512], f32, tag="o", name="outq0", bufs=1)
            for qc in range(1, NQ):
                for n4 in range(0, NK, 4):
                    for n in range(n4, n4 + 4):
                        emit_score(qc, n)
                    pump_pv(qc - 1, n4 + 4)
                    if qc == NQ - 1:
                        ge = sum(1 for g in range(6) if (qc, g) in pt_tiles)
                        if ge >= 3:
                            if outqs[qc] is None:
                                outqs[qc] = ps.tile(
                                    [H + 1, 512],
                                    f32,
                                    tag=("o", "pj")[qc % 2],
                                    name=f"outq{qc}",
                                    bufs=1,
                                )
                            pump_pv(qc, 3 * (ge - 2))
                emit_epilogue(qc - 1)
                if qc < NQ - 1:
                    outqs[qc] = ps.tile(
                        [H + 1, 512],
                        f32,
                        tag=("o", "pj")[qc % 2],
                        name=f"outq{qc}",
                        bufs=1,
                    )
            pump_pv(NQ - 1, NK)
            emit_epilogue(NQ - 1)

    nc.compile()
    return nc


def _get_nc():
    if "nc" not in _cache:
        _cache["nc"] = _build()
    return _cache["nc"]


def _prep_inputs(x, Wq, bq, Wk, bk, Wv, bv):
    import ml_dtypes

    x = np.asarray(x, np.float32)
    Wq = np.asarray(Wq, np.float32)
    Wk = np.asarray(Wk, np.float32)
    Wv = np.asarray(Wv, np.float32)
    bq = np.asarray(bq, np.float32).ravel()
    bk = np.asarray(bk, np.float32).ravel()
    bv = np.asarray(bv, np.float32).ravel()

    def wprep(w, m):  # [768,m] -> [128, DT*m]: (p, dt*m+h) = w[dt*128+p, h]
        return np.ascontiguousarray(
            w.reshape(DT, 128, m).transpose(1, 0, 2).reshape(128, DT * m)
        ).astype(ml_dtypes.bfloat16)

    common = {
        "wA": wprep(np.concatenate([Wv, Wq], axis=1), 128),
        "wB": wprep(Wk, H),
        "bA": np.ascontiguousarray(np.concatenate([bv, bq]).reshape(128, 1)),
        "bB": np.ascontiguousarray(
            np.concatenate([np.zeros(H, np.float32), bk]).reshape(128, 1)
        ),
    }
    return x, common


def _xprep(xb):
    """[S, D] f32 -> [128, NQ, 2, DT*256] bf16:
    (p, c, h, dt*256+j) = x[c*512 + h*256 + j, dt*128 + p]"""
    import ml_dtypes

    t = xb.reshape(NQ, 2, 256, DT, 128).transpose(4, 0, 1, 3, 2)
    return np.ascontiguousarray(t.reshape(128, NQ, 2, DT * 256)).astype(
        ml_dtypes.bfloat16
    )


def _unshard_out(o):
    """[65, NQ*512] out^T with denominator row -> [S, H]"""
    o = np.asarray(o, np.float32)
    return (o[:H, :] / o[H : H + 1, :]).T


def _in_maps(x, common):
    return [{"xp": _xprep(x[b]), **common} for b in range(B)]


def kernel(x, Wq, bq, Wk, bk, Wv, bv, **_):
    from concourse.bass_utils import run_bass_kernel_spmd

    nc = _get_nc()
    x, common = _prep_inputs(x, Wq, bq, Wk, bk, Wv, bv)
    res = run_bass_kernel_spmd(nc, _in_maps(x, common), core_ids=list(range(B)))
    return np.stack([_unshard_out(res.results[b]["out"]) for b in range(B)])


# revision 53
# speedup vs baseline: 1.0300x; 1.0300x over previous
"""Single-head attention on 8 Trainium2 NeuronCores, batch-sharded.

Per core (one batch element b). Host-side layouts make every DMA a large
contiguous read (3KB/partition halves of x chunks).

Projections (bf16, chunk order 0..3, two fused groups):
  A [Wv|Wq] -> vq tile: rows 0-63 v^T, rows 64-127 q^T  (one DVE add)
  B [Wk]    -> kT tile rows 64-127 (M=64 matmul, tile_position (0,64));
               kT rows 0-63 are memset to zero once.
Scores then run the K=128 contraction directly on these tiles: the zero
rows of kT annihilate the v rows of vq, so NO zero-padded copies of q are
needed anywhere:  scores^T[kt] = kT[:,kt-cols]^T @ vq[:,q-cols].

Scores: 16 serial matmuls [K=128, M=128, N=512] per q-chunk at the warm
issue rate (~216ns); measured row-tiling of K=64 pairs gives NO speedup
(concurrent row-tiles serialize on the moving-operand SBUF port), so the
simple padded form wins (it needs 6144 fewer projection rows).
q-chunk-0 scores for k-tiles 4c..4c+3 are emitted right after chunk c's
projections, so the exp stream starts ~3us into the projection phase.

exp: split across TWO engines writing bf16 P^T tiles from [128,1536]
psum score tiles. ACT handles most groups (exact exp, scale=1/8 folded
in). DVE handles groups 0,3 of q-chunks 1-3 with a Schraudolph bit-trick:
bf16 bits of 2^y are linear in y, so
    bits = round(s_raw * (log2e/8 * 128) + 16248.5)
computed by ONE tensor_scalar (mult+add, f32 psum in, int16 out) IS
exp(s/8) to within ~2%; the int16 tile is bitcast to bf16 for the PV
matmul. This removes exp as the serial bottleneck (sim rel err of the
mix: 0.7e-2 < 2e-2 budget).

PV (bf16): per k-tile matmul, M=65 (V plus a ones row -> softmax
denominator row), accumulated over 16 k-tiles into a 1-bank psum.
PV(qc-1) interleaves with scores(qc); PV(3) trails its own exps by two
groups inside qc=3 to shorten the tail. outq psum banks alternate
between the "o" and "pj" tags (which projections also rotate through
during the projection phase).

V layout: per-chunk PE transposes of vq rows into v65 [128, kt, 80]
(pitch 160B) + DVE copy; a DMA-transpose variant exists but measured
slower (queue issue cost) — kept behind USE_DMA_TRANSPOSE.

Epilogue per q-chunk: DVE copy psum->SBUF, DMA out^T [65,512] f32; the
host does the divide-by-denominator and the final transpose (cheap).

PSUM: tag "sc" 2x3 banks (score tiles), tags "pj"+"o" 1 bank each
(projection psums, V-transpose staging, outq accumulators) = 8 banks.
"""

import numpy as np

USE_DMA_TRANSPOSE = False
VPITCH = 80  # v65 per-k-tile pitch: 160B — every multiple is 32B-aligned
             # (DMA-transpose dest requires 32B alignment)

B, S, D, H = 8, 2048, 768, 64
DT = D // 128          # 6 d-tiles
NQ = S // 512          # 4 q-chunks of 512
NK = S // 128          # 16 k-tiles of 128
SCALE = 1.0 / np.sqrt(H).item()
SCH_A = SCALE * np.log2(np.e).item() * 128.0   # Schraudolph slope
SCH_C = 16248.5                                 # Schraudolph offset (tuned)
GROUPS = ((0, 3), (3, 6), (6, 9), (9, 12), (12, 15), (15, 16))
DVE_GROUPS = {(qc, g) for qc in (1, 2, 3) for g in (0, 3)}

_cache = {}


def _build():
    import concourse.mybir as mybir
    import concourse.tile as tile
    from concourse import bacc
    from concourse.masks import make_identity

    f32 = mybir.dt.float32
    bf16 = mybir.dt.bfloat16
    i16 = mybir.dt.int16
    Exp = mybir.ActivationFunctionType.Exp
    Mult = mybir.AluOpType.mult
    Add = mybir.AluOpType.add

    nc = bacc.Bacc(None)
    xp_d = nc.dram_tensor("xp", [128, NQ, 2, DT * 256], bf16, kind="ExternalInput")
    wA_d = nc.dram_tensor("wA", [128, DT * 128], bf16, kind="ExternalInput")
    wB_d = nc.dram_tensor("wB", [128, DT * 64], bf16, kind="ExternalInput")
    bA_d = nc.dram_tensor("bA", [128, 1], f32, kind="ExternalInput")
    bB_d = nc.dram_tensor("bB", [128, 1], f32, kind="ExternalInput")
    out_d = nc.dram_tensor("out", [H + 1, NQ * 512], f32, kind="ExternalOutput")

    with tile.TileContext(nc) as tc:
        with (
            tc.tile_pool(name="big", bufs=1) as big,
            tc.tile_pool(name="small", bufs=1) as small,
            tc.tile_pool(name="pt", bufs=10) as ptp,
            tc.tile_pool(name="res", bufs=2) as resp,
            tc.tile_pool(name="ps", bufs=2, space="PSUM") as ps,
        ):
            # ---- PE warmup: matmuls on a DVE-memset tile during the DMA
            # fill push the HAM clock gate to K=8/8 before real work ----
            wsrc = small.tile([128, 128], bf16)
            nc.vector.memset(wsrc, 1.0)
            wps = ps.tile([128, 128], f32, tag="pj", name="warmps", bufs=1)
            for i in range(36):
                nc.tensor.matmul(wps, lhsT=wsrc, rhs=wsrc, start=True, stop=True)

            if not USE_DMA_TRANSPOSE:
                ident = small.tile([128, 128], f32)
                make_identity(nc, ident)
                identb = small.tile([128, 128], bf16)
                nc.gpsimd.tensor_copy(out=identb, in_=ident)

            # warm the ACT exp table during DMA fill
            zwarm = small.tile([128, 8], f32)
            nc.gpsimd.memset(zwarm, 0.0)
            wwarm = small.tile([128, 8], bf16)
            nc.scalar.activation(out=wwarm, in_=zwarm, func=Exp)

            # ---- DMAs, ordered so chunk-0 compute starts earliest ----
            wA = small.tile([128, DT, 128], bf16)
            wB = small.tile([128, DT, H], bf16)
            bA = small.tile([128, 1], f32)
            bB = small.tile([128, 1], f32)
            xT = big.tile([128, NQ, 2, DT * 256], bf16)

            nc.sync.dma_start(out=wA, in_=wA_d[:, :].rearrange("p (t h) -> p t h", t=DT))
            nc.sync.dma_start(out=wB, in_=wB_d[:, :].rearrange("p (t h) -> p t h", t=DT))
            nc.sync.dma_start(out=bA, in_=bA_d[:, :])
            nc.sync.dma_start(out=bB, in_=bB_d[:, :])
            for c in range(4):
                for hh in (0, 1):
                    nc.sync.dma_start(out=xT[:, c, hh, :], in_=xp_d[:, c, hh, :])

            # ---- PE warmup: dummy matmuls during the DMA fill get the HAM
            # clock gate to K=8/8 (2.4 GHz) before the first real matmul ----
            # ---- persistent tensors ----
            vq = big.tile([128, S], bf16, tag="vq")   # v^T lo / q^T hi
            kT = big.tile([128, S], bf16, tag="kT")   # zeros lo / k^T hi
            nc.gpsimd.memset(kT[:H, :], 0.0)
            v65 = big.tile([128, NK, VPITCH], bf16, tag="v65")
            nc.gpsimd.memset(v65[:, :, H : H + 1], 1.0)

            # proj psums / vtrans staging / outq alternate two 1-bank tags
            alt = {"n": 0}

            def pj_tile(cols, nm, dtype=f32):
                tag = ("pj", "o")[alt["n"] % 2]
                alt["n"] += 1
                return ps.tile([128, cols], dtype, tag=tag, name=nm, bufs=1)

            pt_tiles = {}
            sc_state = {}
            outqs = [None] * NQ

            def emit_pv(qc, n):
                g = n // 3
                lo, _hi = GROUPS[g]
                pt = pt_tiles[(qc, g)]
                rhs_t = pt if pt.dtype == bf16 else pt.bitcast(bf16)
                slot = n - lo
                nc.tensor.matmul(
                    outqs[qc],
                    lhsT=v65[:, n, : H + 1],
                    rhs=rhs_t[:, slot * 512 : (slot + 1) * 512],
                    start=(n == 0),
                    stop=(n == NK - 1),
                )

            def emit_epilogue(qc):
                oT = resp.tile([H + 1, 512], f32, tag="oT", name=f"oT{qc}")
                nc.vector.tensor_copy(out=oT, in_=outqs[qc])
                nc.sync.dma_start(
                    out=out_d[:, qc * 512 : (qc + 1) * 512], in_=oT
                )

            def emit_score(qc, n):
                """scores^T for k-tile n against q-chunk qc (K=128 padded)."""
                st = sc_state.setdefault(qc, [None] * len(GROUPS))
                g = n // 3
                lo, hi = GROUPS[g]
                if st[g] is None:
                    st[g] = ps.tile(
                        [128, (hi - lo) * 512], f32, tag="sc", name=f"sc{qc}_{g}"
                    )
                slot = n - lo
                nc.tensor.matmul(
                    st[g][:, slot * 512 : (slot + 1) * 512],
                    lhsT=kT[:, n * 128 : (n + 1) * 128],
                    rhs=vq[:, qc * 512 : (qc + 1) * 512],
                    start=True,
                    stop=True,
                )
                if n == hi - 1:  # group full -> exp
                    cols = (hi - lo) * 512
                    if (qc, g) in DVE_GROUPS:
                        pt = ptp.tile([128, cols], i16, tag="pT", name=f"pt{qc}_{g}")
                        nc.vector.tensor_scalar(
                            out=pt,
                            in0=st[g],
                            scalar1=SCH_A,
                            scalar2=SCH_C,
                            op0=Mult,
                            op1=Add,
                        )
                    else:
                        pt = ptp.tile([128, cols], bf16, tag="pT", name=f"pt{qc}_{g}")
                        nc.scalar.activation(out=pt, in_=st[g], func=Exp, scale=SCALE)
                    pt_tiles[(qc, g)] = pt

            # ---- projection phase ----
            # Dependent work for chunk c (V transposes, q-chunk-0 scores)
            # is emitted AFTER chunk c+1's projection matmuls, so the DVE
            # bias-adds have a full matmul group's time to complete.
            def chunk_tail(c):
                for j in range(4):
                    kt = 4 * c + j
                    if USE_DMA_TRANSPOSE:
                        nc.sync.dma_start(
                            out=v65[:, kt, :H],
                            in_=vq[:H, kt * 128 : (kt + 1) * 128],
                            transpose=True,
                        )
                    else:
                        tp = pj_tile(128, f"vtr{kt}", dtype=bf16)
                        nc.tensor.transpose(
                            tp[:, :H],
                            vq[:H, kt * 128 : (kt + 1) * 128],
                            identb[:H, :H],
                        )
                        nc.vector.tensor_copy(out=v65[:, kt, :H], in_=tp[:, :H])
                for j in range(4):
                    emit_score(0, 4 * c + j)

            for c in range(4):
                cc = slice(c * 512, (c + 1) * 512)

                def proj(w, nm, mlo, mhi, c0=c):
                    p = pj_tile(512, f"ps{nm}{c0}")
                    for hh in (0, 1):
                        for dt in range(DT):
                            nc.tensor.matmul(
                                p[mlo:mhi, hh * 256 : (hh + 1) * 256],
                                lhsT=w[:, dt, :],
                                rhs=xT[:, c0, hh, dt * 256 : (dt + 1) * 256],
                                start=(dt == 0),
                                stop=(dt == DT - 1),
                            )
                    return p

                psA = proj(wA, "A", 0, 128)
                nc.vector.tensor_scalar_add(out=vq[:, cc], in0=psA, scalar1=bA)
                psB = proj(wB, "B", H, 128)
                nc.vector.tensor_scalar_add(
                    out=kT[H:, cc], in0=psB[H:, :], scalar1=bB[H:, :]
                )
                if c >= 1:
                    chunk_tail(c - 1)
            chunk_tail(3)

            # ---- steady phase ----
            pv_cursor = [0] * NQ

            def pump_pv(qc, limit_n):
                while pv_cursor[qc] < min(limit_n, NK):
                    emit_pv(qc, pv_cursor[qc])
                    pv_cursor[qc] += 1

            outqs[0] = ps.tile([H + 1, 512], f32, tag="o", name="outq0", bufs=1)
            for qc in range(1, NQ):
                for n4 in range(0, NK, 4):
                    for n in range(n4, n4 + 4):
                        emit_score(qc, n)
                    pump_pv(qc - 1, n4 + 4)
                    if qc == NQ - 1:
                        ge = sum(1 for g in range(6) if (qc, g) in pt_tiles)
                        if ge >= 3:
                            if outqs[qc] is None:
                                outqs[qc] = ps.tile(
                                    [H + 1, 512],
                                    f32,
                                    tag=("o", "pj")[qc % 2],
                                    name=f"outq{qc}",
                                    bufs=1,
                                )
                            pump_pv(qc, 3 * (ge - 2))
                emit_epilogue(qc - 1)
                if qc < NQ - 1:
                    outqs[qc] = ps.tile(
                        [H + 1, 512],
                        f32,
                        tag=("o", "pj")[qc % 2],
                        name=f"outq{qc}",
                        bufs=1,
                    )
            pump_pv(NQ - 1, NK)
            emit_epilogue(NQ - 1)

    nc.compile()
    return nc


def _get_nc():
    if "nc" not in _cache:
        _cache["nc"] = _build()
    return _cache["nc"]


def _prep_inputs(x, Wq, bq, Wk, bk, Wv, bv):
    import ml_dtypes

    x = np.asarray(x, np.float32)
    Wq = np.asarray(Wq, np.float32)
    Wk = np.asarray(Wk, np.float32)
    Wv = np.asarray(Wv, np.float32)
    bq = np.asarray(bq, np.float32).ravel()
    bk = np.asarray(bk, np.float32).ravel()
    bv = np.asarray(bv, np.float32).ravel()

    def wprep(w, m):  # [768,m] -> [128, DT*m]: (p, dt*m+h) = w[dt*128+p, h]
        return np.ascontiguousarray(
            w.reshape(DT, 128, m).transpose(1, 0, 2).reshape(128, DT * m)
        ).astype(ml_dtypes.bfloat16)

    common = {
        "wA": wprep(np.concatenate([Wv, Wq], axis=1), 128),
        "wB": wprep(Wk, H),
        "bA": np.ascontiguousarray(np.concatenate([bv, bq]).reshape(128, 1)),
        "bB": np.ascontiguousarray(
            np.concatenate([np.zeros(H, np.float32), bk]).reshape(128, 1)
        ),
    }
    return x, common


def _xprep(xb):
    """[S, D] f32 -> [128, NQ, 2, DT*256] bf16:
    (p, c, h, dt*256+j) = x[c*512 + h*256 + j, dt*128 + p]"""
    import ml_dtypes

    t = xb.reshape(NQ, 2, 256, DT, 128).transpose(4, 0, 1, 3, 2)
    return np.ascontiguousarray(t.reshape(128, NQ, 2, DT * 256)).astype(
        ml_dtypes.bfloat16
    )


def _unshard_out(o):
    """[65, NQ*512] out^T with denominator row -> [S, H]"""
    o = np.asarray(o, np.float32)
    return (o[:H, :] / o[H : H + 1, :]).T


def _in_maps(x, common):
    return [{"xp": _xprep(x[b]), **common} for b in range(B)]


def kernel(x, Wq, bq, Wk, bk, Wv, bv, **_):
    from concourse.bass_utils import run_bass_kernel_spmd

    nc = _get_nc()
    x, common = _prep_inputs(x, Wq, bq, Wk, bk, Wv, bv)
    res = run_bass_kernel_spmd(nc, _in_maps(x, common), core_ids=list(range(B)))
    return np.stack([_unshard_out(res.results[b]["out"]) for b in range(B)])


# revision 55
# speedup vs baseline: 1.0458x; 1.0154x over previous
"""Single-head attention on 8 Trainium2 NeuronCores, batch-sharded.

Per core (one batch element b). Host-side layouts make every DMA a large
contiguous read (3KB/partition halves of x chunks).

Projections (bf16, chunk order 0..3, two fused groups):
  A [Wv|Wq] -> vq tile: rows 0-63 v^T, rows 64-127 q^T  (one DVE add)
  B [Wk]    -> kT tile rows 64-127 (M=64 matmul, tile_position (0,64));
               kT rows 0-63 are memset to zero once.
Scores then run the K=128 contraction directly on these tiles: the zero
rows of kT annihilate the v rows of vq, so NO zero-padded copies of q are
needed anywhere:  scores^T[kt] = kT[:,kt-cols]^T @ vq[:,q-cols].

Scores: 16 serial matmuls [K=128, M=128, N=512] per q-chunk at the warm
issue rate (~216ns); measured row-tiling of K=64 pairs gives NO speedup
(concurrent row-tiles serialize on the moving-operand SBUF port), so the
simple padded form wins (it needs 6144 fewer projection rows).
q-chunk-0 scores for k-tiles 4c..4c+3 are emitted right after chunk c's
projections, so the exp stream starts ~3us into the projection phase.

exp: split across TWO engines writing bf16 P^T tiles from [128,1536]
psum score tiles. ACT handles most groups (exact exp, scale=1/8 folded
in). DVE handles groups 0,3 of q-chunks 1-3 with a Schraudolph bit-trick:
bf16 bits of 2^y are linear in y, so
    bits = round(s_raw * (log2e/8 * 128) + 16248.5)
computed by ONE tensor_scalar (mult+add, f32 psum in, int16 out) IS
exp(s/8) to within ~2%; the int16 tile is bitcast to bf16 for the PV
matmul. This removes exp as the serial bottleneck (sim rel err of the
mix: 0.7e-2 < 2e-2 budget).

PV (bf16): per k-tile matmul, M=65 (V plus a ones row -> softmax
denominator row), accumulated over 16 k-tiles into a 1-bank psum.
PV(qc-1) interleaves with scores(qc); PV(3) trails its own exps by two
groups inside qc=3 to shorten the tail. outq psum banks alternate
between the "o" and "pj" tags (which projections also rotate through
during the projection phase).

V layout: per-chunk PE transposes of vq rows into v65 [128, kt, 80]
(pitch 160B) + DVE copy; a DMA-transpose variant exists but measured
slower (queue issue cost) — kept behind USE_DMA_TRANSPOSE.

Epilogue per q-chunk: DVE copy psum->SBUF, DMA out^T [65,512] f32; the
host does the divide-by-denominator and the final transpose (cheap).

PSUM: tag "sc" 2x3 banks (score tiles), tags "pj"+"o" 1 bank each
(projection psums, V-transpose staging, outq accumulators) = 8 banks.
"""

import numpy as np

USE_DMA_TRANSPOSE = False
VPITCH = 80  # v65 per-k-tile pitch: 160B — every multiple is 32B-aligned
             # (DMA-transpose dest requires 32B alignment)

B, S, D, H = 8, 2048, 768, 64
DT = D // 128          # 6 d-tiles
NQ = S // 512          # 4 q-chunks of 512
NK = S // 128          # 16 k-tiles of 128
SCALE = 1.0 / np.sqrt(H).item()
SCH_A = SCALE * np.log2(np.e).item() * 128.0   # Schraudolph slope
SCH_C = 16248.5                                 # Schraudolph offset (tuned)
GROUPS = ((0, 3), (3, 6), (6, 9), (9, 12), (12, 15), (15, 16))
DVE_GROUPS = {(qc, g) for qc in (1, 2, 3) for g in (0, 3)}

_cache = {}


def _build():
    import concourse.mybir as mybir
    import concourse.tile as tile
    from concourse import bacc
    from concourse.masks import make_identity

    f32 = mybir.dt.float32
    bf16 = mybir.dt.bfloat16
    i16 = mybir.dt.int16
    Exp = mybir.ActivationFunctionType.Exp
    Mult = mybir.AluOpType.mult
    Add = mybir.AluOpType.add

    nc = bacc.Bacc(None)
    xp_d = nc.dram_tensor("xp", [128, NQ, 2, DT * 256], bf16, kind="ExternalInput")
    wA_d = nc.dram_tensor("wA", [128, DT * 128], bf16, kind="ExternalInput")
    wB_d = nc.dram_tensor("wB", [128, DT * 64], bf16, kind="ExternalInput")
    bA_d = nc.dram_tensor("bA", [128, 1], f32, kind="ExternalInput")
    bB_d = nc.dram_tensor("bB", [128, 1], f32, kind="ExternalInput")
    out_d = nc.dram_tensor("out", [H + 1, NQ * 512], f32, kind="ExternalOutput")

    with tile.TileContext(nc) as tc:
        with (
            tc.tile_pool(name="big", bufs=1) as big,
            tc.tile_pool(name="small", bufs=1) as small,
            tc.tile_pool(name="pt", bufs=10) as ptp,
            tc.tile_pool(name="res", bufs=2) as resp,
            tc.tile_pool(name="ps", bufs=2, space="PSUM") as ps,
        ):
            # ---- PE warmup: matmuls on a DVE-memset tile during the DMA
            # fill push the HAM clock gate to K=8/8 before real work ----
            wsrc = small.tile([128, 128], bf16)
            nc.vector.memset(wsrc, 1.0)
            wps = ps.tile([128, 128], f32, tag="pj", name="warmps", bufs=1)
            for i in range(44):
                nc.tensor.matmul(wps, lhsT=wsrc, rhs=wsrc, start=True, stop=True)

            if not USE_DMA_TRANSPOSE:
                ident = small.tile([128, 128], f32)
                make_identity(nc, ident)
                identb = small.tile([128, 128], bf16)
                nc.gpsimd.tensor_copy(out=identb, in_=ident)

            # warm the ACT exp table during DMA fill
            zwarm = small.tile([128, 8], f32)
            nc.gpsimd.memset(zwarm, 0.0)
            wwarm = small.tile([128, 8], bf16)
            nc.scalar.activation(out=wwarm, in_=zwarm, func=Exp)

            # ---- DMAs, ordered so chunk-0 compute starts earliest ----
            wA = small.tile([128, DT, 128], bf16)
            wB = small.tile([128, DT, H], bf16)
            bA = small.tile([128, 1], f32)
            bB = small.tile([128, 1], f32)
            xT = big.tile([128, NQ, 2, DT * 256], bf16)

            nc.sync.dma_start(out=wA, in_=wA_d[:, :].rearrange("p (t h) -> p t h", t=DT))
            nc.sync.dma_start(out=bA, in_=bA_d[:, :])
            nc.sync.dma_start(out=xT[:, 0, 0, :], in_=xp_d[:, 0, 0, :])
            nc.sync.dma_start(out=xT[:, 0, 1, :], in_=xp_d[:, 0, 1, :])
            nc.sync.dma_start(out=wB, in_=wB_d[:, :].rearrange("p (t h) -> p t h", t=DT))
            nc.sync.dma_start(out=bB, in_=bB_d[:, :])
            for c in (1, 2, 3):
                for hh in (0, 1):
                    nc.sync.dma_start(out=xT[:, c, hh, :], in_=xp_d[:, c, hh, :])

            # ---- PE warmup: dummy matmuls during the DMA fill get the HAM
            # clock gate to K=8/8 (2.4 GHz) before the first real matmul ----
            # ---- persistent tensors ----
            vq = big.tile([128, S], bf16, tag="vq")   # v^T lo / q^T hi
            kT = big.tile([128, S], bf16, tag="kT")   # zeros lo / k^T hi
            nc.gpsimd.memset(kT[:H, :], 0.0)
            v65 = big.tile([128, NK, VPITCH], bf16, tag="v65")
            nc.gpsimd.memset(v65[:, :, H : H + 1], 1.0)

            # proj psums / vtrans staging / outq alternate two 1-bank tags
            alt = {"n": 0}

            def pj_tile(cols, nm, dtype=f32):
                tag = ("pj", "o")[alt["n"] % 2]
                alt["n"] += 1
                return ps.tile([128, cols], dtype, tag=tag, name=nm, bufs=1)

            pt_tiles = {}
            sc_state = {}
            outqs = [None] * NQ

            def emit_pv(qc, n):
                g = n // 3
                lo, _hi = GROUPS[g]
                pt = pt_tiles[(qc, g)]
                rhs_t = pt if pt.dtype == bf16 else pt.bitcast(bf16)
                slot = n - lo
                nc.tensor.matmul(
                    outqs[qc],
                    lhsT=v65[:, n, : H + 1],
                    rhs=rhs_t[:, slot * 512 : (slot + 1) * 512],
                    start=(n == 0),
                    stop=(n == NK - 1),
                )

            def emit_epilogue(qc):
                oT = resp.tile([H + 1, 512], f32, tag="oT", name=f"oT{qc}")
                nc.vector.tensor_copy(out=oT, in_=outqs[qc])
                nc.sync.dma_start(
                    out=out_d[:, qc * 512 : (qc + 1) * 512], in_=oT
                )

            def emit_score(qc, n):
                """scores^T for k-tile n against q-chunk qc (K=128 padded)."""
                st = sc_state.setdefault(qc, [None] * len(GROUPS))
                g = n // 3
                lo, hi = GROUPS[g]
                if st[g] is None:
                    st[g] = ps.tile(
                        [128, (hi - lo) * 512], f32, tag="sc", name=f"sc{qc}_{g}"
                    )
                slot = n - lo
                nc.tensor.matmul(
                    st[g][:, slot * 512 : (slot + 1) * 512],
                    lhsT=kT[:, n * 128 : (n + 1) * 128],
                    rhs=vq[:, qc * 512 : (qc + 1) * 512],
                    start=True,
                    stop=True,
                )
                if n == hi - 1:  # group full -> exp
                    cols = (hi - lo) * 512
                    if (qc, g) in DVE_GROUPS:
                        pt = ptp.tile([128, cols], i16, tag="pT", name=f"pt{qc}_{g}")
                        nc.vector.tensor_scalar(
                            out=pt,
                            in0=st[g],
                            scalar1=SCH_A,
                            scalar2=SCH_C,
                            op0=Mult,
                            op1=Add,
                        )
                    else:
                        pt = ptp.tile([128, cols], bf16, tag="pT", name=f"pt{qc}_{g}")
                        nc.scalar.activation(out=pt, in_=st[g], func=Exp, scale=SCALE)
                    pt_tiles[(qc, g)] = pt

            # ---- projection phase ----
            # Dependent work for chunk c (V transposes, q-chunk-0 scores)
            # is emitted AFTER chunk c+1's projection matmuls, so the DVE
            # bias-adds have a full matmul group's time to complete.
            def chunk_tail(c):
                for j in range(4):
                    kt = 4 * c + j
                    if USE_DMA_TRANSPOSE:
                        nc.sync.dma_start(
                            out=v65[:, kt, :H],
                            in_=vq[:H, kt * 128 : (kt + 1) * 128],
                            transpose=True,
                        )
                    else:
                        tp = pj_tile(128, f"vtr{kt}", dtype=bf16)
                        nc.tensor.transpose(
                            tp[:, :H],
                            vq[:H, kt * 128 : (kt + 1) * 128],
                            identb[:H, :H],
                        )
                        nc.vector.tensor_copy(out=v65[:, kt, :H], in_=tp[:, :H])
                for j in range(4):
                    emit_score(0, 4 * c + j)

            for c in range(4):
                cc = slice(c * 512, (c + 1) * 512)

                def proj(w, nm, mlo, mhi, c0=c):
                    p = pj_tile(512, f"ps{nm}{c0}")
                    for hh in (0, 1):
                        for dt in range(DT):
                            nc.tensor.matmul(
                                p[mlo:mhi, hh * 256 : (hh + 1) * 256],
                                lhsT=w[:, dt, :],
                                rhs=xT[:, c0, hh, dt * 256 : (dt + 1) * 256],
                                start=(dt == 0),
                                stop=(dt == DT - 1),
                            )
                    return p

                psA = proj(wA, "A", 0, 128)
                nc.vector.tensor_scalar_add(out=vq[:, cc], in0=psA, scalar1=bA)
                psB = proj(wB, "B", H, 128)
                nc.vector.tensor_scalar_add(
                    out=kT[H:, cc], in0=psB[H:, :], scalar1=bB[H:, :]
                )
                if c >= 1:
                    chunk_tail(c - 1)
            chunk_tail(3)

            # ---- steady phase ----
            pv_cursor = [0] * NQ

            def pump_pv(qc, limit_n):
                while pv_cursor[qc] < min(limit_n, NK):
                    emit_pv(qc, pv_cursor[qc])
                    pv_cursor[qc] += 1

            outqs[0] = ps.tile([H + 1, 512], f32, tag="o", name="outq0", bufs=1)
            for qc in range(1, NQ):
                for n4 in range(0, NK, 4):
                    for n in range(n4, n4 + 4):
                        emit_score(qc, n)
                    pump_pv(qc - 1, n4 + 4)
                    if qc == NQ - 1:
                        ge = sum(1 for g in range(6) if (qc, g) in pt_tiles)
                        if ge >= 3:
                            if outqs[qc] is None:
                                outqs[qc] = ps.tile(
                                    [H + 1, 512],
                                    f32,
                                    tag=("o", "pj")[qc % 2],
                                    name=f"outq{qc}",
                                    bufs=1,
                                )
                            pump_pv(qc, 3 * (ge - 2))
                emit_epilogue(qc - 1)
                if qc < NQ - 1:
                    outqs[qc] = ps.tile(
                        [H + 1, 512],
                        f32,
                        tag=("o", "pj")[qc % 2],
                        name=f"outq{qc}",
                        bufs=1,
                    )
            pump_pv(NQ - 1, NK)
            emit_epilogue(NQ - 1)

    nc.compile()
    return nc


def _get_nc():
    if "nc" not in _cache:
        _cache["nc"] = _build()
    return _cache["nc"]


def _prep_inputs(x, Wq, bq, Wk, bk, Wv, bv):
    import ml_dtypes

    x = np.asarray(x, np.float32)
    Wq = np.asarray(Wq, np.float32)
    Wk = np.asarray(Wk, np.float32)
    Wv = np.asarray(Wv, np.float32)
    bq = np.asarray(bq, np.float32).ravel()
    bk = np.asarray(bk, np.float32).ravel()
    bv = np.asarray(bv, np.float32).ravel()

    def wprep(w, m):  # [768,m] -> [128, DT*m]: (p, dt*m+h) = w[dt*128+p, h]
        return np.ascontiguousarray(
            w.reshape(DT, 128, m).transpose(1, 0, 2).reshape(128, DT * m)
        ).astype(ml_dtypes.bfloat16)

    common = {
        "wA": wprep(np.concatenate([Wv, Wq], axis=1), 128),
        "wB": wprep(Wk, H),
        "bA": np.ascontiguousarray(np.concatenate([bv, bq]).reshape(128, 1)),
        "bB": np.ascontiguousarray(
            np.concatenate([np.zeros(H, np.float32), bk]).reshape(128, 1)
        ),
    }
    return x, common


def _xprep(xb):
    """[S, D] f32 -> [128, NQ, 2, DT*256] bf16:
    (p, c, h, dt*256+j) = x[c*512 + h*256 + j, dt*128 + p]"""
    import ml_dtypes

    t = xb.reshape(NQ, 2, 256, DT, 128).transpose(4, 0, 1, 3, 2)
    return np.ascontiguousarray(t.reshape(128, NQ, 2, DT * 256)).astype(
        ml_dtypes.bfloat16
    )


def _unshard_out(o):
    """[65, NQ*512] out^T with denominator row -> [S, H]"""
    o = np.asarray(o, np.float32)
    return (o[:H, :] / o[H : H + 1, :]).T


def _in_maps(x, common):
    return [{"xp": _xprep(x[b]), **common} for b in range(B)]


def kernel(x, Wq, bq, Wk, bk, Wv, bv, **_):
    from concourse.bass_utils import run_bass_kernel_spmd

    nc = _get_nc()
    x, common = _prep_inputs(x, Wq, bq, Wk, bk, Wv, bv)
    res = run_bass_kernel_spmd(nc, _in_maps(x, common), core_ids=list(range(B)))
    return np.stack([_unshard_out(res.results[b]["out"]) for b in range(B)])


# revision 58
# speedup vs baseline: 1.0518x; 1.0057x over previous
"""Single-head attention on 8 Trainium2 NeuronCores, batch-sharded.

Per core (one batch element b). Host-side layouts make every DMA a large
contiguous read (3KB/partition halves of x chunks).

Projections (bf16, chunk order 0..3, two fused groups):
  A [Wv|Wq] -> vq tile: rows 0-63 v^T, rows 64-127 q^T  (one DVE add)
  B [Wk]    -> kT tile rows 64-127 (M=64 matmul, tile_position (0,64));
               kT rows 0-63 are memset to zero once.
Scores then run the K=128 contraction directly on these tiles: the zero
rows of kT annihilate the v rows of vq, so NO zero-padded copies of q are
needed anywhere:  scores^T[kt] = kT[:,kt-cols]^T @ vq[:,q-cols].

Scores: 16 serial matmuls [K=128, M=128, N=512] per q-chunk at the warm
issue rate (~216ns); measured row-tiling of K=64 pairs gives NO speedup
(concurrent row-tiles serialize on the moving-operand SBUF port), so the
simple padded form wins (it needs 6144 fewer projection rows).
q-chunk-0 scores for k-tiles 4c..4c+3 are emitted right after chunk c's
projections, so the exp stream starts ~3us into the projection phase.

exp: split across TWO engines writing bf16 P^T tiles from [128,1536]
psum score tiles. ACT handles most groups (exact exp, scale=1/8 folded
in). DVE handles groups 0,3 of q-chunks 1-3 with a Schraudolph bit-trick:
bf16 bits of 2^y are linear in y, so
    bits = round(s_raw * (log2e/8 * 128) + 16248.5)
computed by ONE tensor_scalar (mult+add, f32 psum in, int16 out) IS
exp(s/8) to within ~2%; the int16 tile is bitcast to bf16 for the PV
matmul. This removes exp as the serial bottleneck (sim rel err of the
mix: 0.7e-2 < 2e-2 budget).

PV (bf16): per k-tile matmul, M=65 (V plus a ones row -> softmax
denominator row), accumulated over 16 k-tiles into a 1-bank psum.
PV(qc-1) interleaves with scores(qc); PV(3) trails its own exps by two
groups inside qc=3 to shorten the tail. outq psum banks alternate
between the "o" and "pj" tags (which projections also rotate through
during the projection phase).

V layout: per-chunk PE transposes of vq rows into v65 [128, kt, 80]
(pitch 160B) + DVE copy; a DMA-transpose variant exists but measured
slower (queue issue cost) — kept behind USE_DMA_TRANSPOSE.

Epilogue per q-chunk: DVE copy psum->SBUF, DMA out^T [65,512] f32; the
host does the divide-by-denominator and the final transpose (cheap).

PSUM: tag "sc" 2x3 banks (score tiles), tags "pj"+"o" 1 bank each
(projection psums, V-transpose staging, outq accumulators) = 8 banks.
"""

import numpy as np

USE_DMA_TRANSPOSE = False
VPITCH = 80  # v65 per-k-tile pitch: 160B — every multiple is 32B-aligned
             # (DMA-transpose dest requires 32B alignment)

B, S, D, H = 8, 2048, 768, 64
DT = D // 128          # 6 d-tiles
NQ = S // 512          # 4 q-chunks of 512
NK = S // 128          # 16 k-tiles of 128
SCALE = 1.0 / np.sqrt(H).item()
SCH_A = SCALE * np.log2(np.e).item() * 128.0   # Schraudolph slope
SCH_C = 16248.5                                 # Schraudolph offset (tuned)
GROUPS = ((0, 3), (3, 6), (6, 9), (9, 12), (12, 15), (15, 16))
DVE_GROUPS = {(qc, g) for qc in (1, 2, 3) for g in (0, 3)}

_cache = {}


def _build():
    import concourse.mybir as mybir
    import concourse.tile as tile
    from concourse import bacc
    from concourse.masks import make_identity

    f32 = mybir.dt.float32
    bf16 = mybir.dt.bfloat16
    i16 = mybir.dt.int16
    Exp = mybir.ActivationFunctionType.Exp
    Mult = mybir.AluOpType.mult
    Add = mybir.AluOpType.add

    nc = bacc.Bacc(None)
    xp_d = nc.dram_tensor("xp", [128, NQ, 2, DT * 256], bf16, kind="ExternalInput")
    wA_d = nc.dram_tensor("wA", [128, DT * 128], bf16, kind="ExternalInput")
    wB_d = nc.dram_tensor("wB", [128, DT * 64], bf16, kind="ExternalInput")
    bA_d = nc.dram_tensor("bA", [128, 1], f32, kind="ExternalInput")
    bB_d = nc.dram_tensor("bB", [128, 1], f32, kind="ExternalInput")
    out_d = nc.dram_tensor("out", [H + 1, NQ * 512], f32, kind="ExternalOutput")

    with tile.TileContext(nc) as tc:
        with (
            tc.tile_pool(name="big", bufs=1) as big,
            tc.tile_pool(name="small", bufs=1) as small,
            tc.tile_pool(name="pt", bufs=10) as ptp,
            tc.tile_pool(name="res", bufs=2) as resp,
            tc.tile_pool(name="ps", bufs=2, space="PSUM") as ps,
        ):
            # ---- PE warmup: matmuls on a DVE-memset tile during the DMA
            # fill push the HAM clock gate to K=8/8 before real work ----
            wsrc = small.tile([128, 128], bf16)
            nc.vector.memset(wsrc, 1.0)
            wps = ps.tile([128, 128], f32, tag="pj", name="warmps", bufs=1)
            for i in range(52):
                nc.tensor.matmul(wps, lhsT=wsrc, rhs=wsrc, start=True, stop=True)

            if not USE_DMA_TRANSPOSE:
                ident = small.tile([128, 128], f32)
                make_identity(nc, ident)
                identb = small.tile([128, 128], bf16)
                nc.gpsimd.tensor_copy(out=identb, in_=ident)

            # warm the ACT exp table during DMA fill
            zwarm = small.tile([128, 8], f32)
            nc.gpsimd.memset(zwarm, 0.0)
            wwarm = small.tile([128, 8], bf16)
            nc.scalar.activation(out=wwarm, in_=zwarm, func=Exp)

            # ---- DMAs, ordered so chunk-0 compute starts earliest ----
            wA = small.tile([128, DT, 128], bf16)
            wB = small.tile([128, DT, H], bf16)
            bA = small.tile([128, 1], f32)
            bB = small.tile([128, 1], f32)
            xT = big.tile([128, NQ, 2, DT * 256], bf16)

            nc.sync.dma_start(out=wA, in_=wA_d[:, :].rearrange("p (t h) -> p t h", t=DT))
            nc.sync.dma_start(out=bA, in_=bA_d[:, :])
            nc.sync.dma_start(out=xT[:, 0, 0, :], in_=xp_d[:, 0, 0, :])
            nc.sync.dma_start(out=xT[:, 0, 1, :], in_=xp_d[:, 0, 1, :])
            nc.sync.dma_start(out=wB, in_=wB_d[:, :].rearrange("p (t h) -> p t h", t=DT))
            nc.sync.dma_start(out=bB, in_=bB_d[:, :])
            for c in (1, 2, 3):
                for hh in (0, 1):
                    nc.sync.dma_start(out=xT[:, c, hh, :], in_=xp_d[:, c, hh, :])

            # ---- PE warmup: dummy matmuls during the DMA fill get the HAM
            # clock gate to K=8/8 (2.4 GHz) before the first real matmul ----
            # ---- persistent tensors ----
            vq = big.tile([128, S], bf16, tag="vq")   # v^T lo / q^T hi
            kT = big.tile([128, S], bf16, tag="kT")   # zeros lo / k^T hi
            nc.gpsimd.memset(kT[:H, :], 0.0)
            v65 = big.tile([128, NK, VPITCH], bf16, tag="v65")
            nc.gpsimd.memset(v65[:, :, H : H + 1], 1.0)

            # proj psums / vtrans staging / outq alternate two 1-bank tags
            alt = {"n": 0}

            def pj_tile(cols, nm, dtype=f32):
                tag = ("pj", "o")[alt["n"] % 2]
                alt["n"] += 1
                return ps.tile([128, cols], dtype, tag=tag, name=nm, bufs=1)

            pt_tiles = {}
            sc_state = {}
            outqs = [None] * NQ

            def emit_pv(qc, n):
                g = n // 3
                lo, _hi = GROUPS[g]
                pt = pt_tiles[(qc, g)]
                rhs_t = pt if pt.dtype == bf16 else pt.bitcast(bf16)
                slot = n - lo
                nc.tensor.matmul(
                    outqs[qc],
                    lhsT=v65[:, n, : H + 1],
                    rhs=rhs_t[:, slot * 512 : (slot + 1) * 512],
                    start=(n == 0),
                    stop=(n == NK - 1),
                )

            def emit_epilogue(qc, split=False):
                oT = resp.tile([H + 1, 512], f32, tag="oT", name=f"oT{qc}")
                if split:  # final epilogue: overlap the copy with the DMA
                    for hh in (0, 1):
                        nc.vector.tensor_copy(
                            out=oT[:, hh * 256 : (hh + 1) * 256],
                            in_=outqs[qc][:, hh * 256 : (hh + 1) * 256],
                        )
                        nc.sync.dma_start(
                            out=out_d[
                                :, qc * 512 + hh * 256 : qc * 512 + (hh + 1) * 256
                            ],
                            in_=oT[:, hh * 256 : (hh + 1) * 256],
                        )
                else:
                    nc.vector.tensor_copy(out=oT, in_=outqs[qc])
                    nc.sync.dma_start(
                        out=out_d[:, qc * 512 : (qc + 1) * 512], in_=oT
                    )

            def emit_score(qc, n):
                """scores^T for k-tile n against q-chunk qc (K=128 padded)."""
                st = sc_state.setdefault(qc, [None] * len(GROUPS))
                g = n // 3
                lo, hi = GROUPS[g]
                if st[g] is None:
                    st[g] = ps.tile(
                        [128, (hi - lo) * 512], f32, tag="sc", name=f"sc{qc}_{g}"
                    )
                slot = n - lo
                nc.tensor.matmul(
                    st[g][:, slot * 512 : (slot + 1) * 512],
                    lhsT=kT[:, n * 128 : (n + 1) * 128],
                    rhs=vq[:, qc * 512 : (qc + 1) * 512],
                    start=True,
                    stop=True,
                )
                if n == hi - 1:  # group full -> exp
                    cols = (hi - lo) * 512
                    if (qc, g) in DVE_GROUPS:
                        pt = ptp.tile([128, cols], i16, tag="pT", name=f"pt{qc}_{g}")
                        nc.vector.tensor_scalar(
                            out=pt,
                            in0=st[g],
                            scalar1=SCH_A,
                            scalar2=SCH_C,
                            op0=Mult,
                            op1=Add,
                        )
                    else:
                        pt = ptp.tile([128, cols], bf16, tag="pT", name=f"pt{qc}_{g}")
                        nc.scalar.activation(out=pt, in_=st[g], func=Exp, scale=SCALE)
                    pt_tiles[(qc, g)] = pt

            # ---- projection phase ----
            # Dependent work for chunk c (V transposes, q-chunk-0 scores)
            # is emitted AFTER chunk c+1's projection matmuls, so the DVE
            # bias-adds have a full matmul group's time to complete.
            def chunk_tail(c):
                for j in range(4):
                    kt = 4 * c + j
                    if USE_DMA_TRANSPOSE:
                        nc.sync.dma_start(
                            out=v65[:, kt, :H],
                            in_=vq[:H, kt * 128 : (kt + 1) * 128],
                            transpose=True,
                        )
                    else:
                        tp = pj_tile(128, f"vtr{kt}", dtype=bf16)
                        nc.tensor.transpose(
                            tp[:, :H],
                            vq[:H, kt * 128 : (kt + 1) * 128],
                            identb[:H, :H],
                        )
                        nc.vector.tensor_copy(out=v65[:, kt, :H], in_=tp[:, :H])
                for j in range(4):
                    emit_score(0, 4 * c + j)

            for c in range(4):
                cc = slice(c * 512, (c + 1) * 512)

                def proj(w, nm, mlo, mhi, c0=c):
                    p = pj_tile(512, f"ps{nm}{c0}")
                    for hh in (0, 1):
                        for dt in range(DT):
                            nc.tensor.matmul(
                                p[mlo:mhi, hh * 256 : (hh + 1) * 256],
                                lhsT=w[:, dt, :],
                                rhs=xT[:, c0, hh, dt * 256 : (dt + 1) * 256],
                                start=(dt == 0),
                                stop=(dt == DT - 1),
                            )
                    return p

                psA = proj(wA, "A", 0, 128)
                nc.vector.tensor_scalar_add(out=vq[:, cc], in0=psA, scalar1=bA)
                psB = proj(wB, "B", H, 128)
                nc.vector.tensor_scalar_add(
                    out=kT[H:, cc], in0=psB[H:, :], scalar1=bB[H:, :]
                )
                if c >= 1:
                    chunk_tail(c - 1)
            chunk_tail(3)

            # ---- steady phase ----
            pv_cursor = [0] * NQ

            def pump_pv(qc, limit_n):
                while pv_cursor[qc] < min(limit_n, NK):
                    emit_pv(qc, pv_cursor[qc])
                    pv_cursor[qc] += 1

            outqs[0] = ps.tile([H + 1, 512], f32, tag="o", name="outq0", bufs=1)
            for qc in range(1, NQ):
                for n4 in range(0, NK, 4):
                    for n in range(n4, n4 + 4):
                        emit_score(qc, n)
                    pump_pv(qc - 1, n4 + 4)
                    if qc == NQ - 1:
                        ge = sum(1 for g in range(6) if (qc, g) in pt_tiles)
                        if ge >= 3:
                            if outqs[qc] is None:
                                outqs[qc] = ps.tile(
                                    [H + 1, 512],
                                    f32,
                                    tag=("o", "pj")[qc % 2],
                                    name=f"outq{qc}",
                                    bufs=1,
                                )
                            pump_pv(qc, 3 * (ge - 2))
                emit_epilogue(qc - 1)
                if qc < NQ - 1:
                    outqs[qc] = ps.tile(
                        [H + 1, 512],
                        f32,
                        tag=("o", "pj")[qc % 2],
                        name=f"outq{qc}",
                        bufs=1,
                    )
            pump_pv(NQ - 1, NK)
            emit_epilogue(NQ - 1, split=True)

    nc.compile()
    return nc


def _get_nc():
    if "nc" not in _cache:
        _cache["nc"] = _build()
    return _cache["nc"]


def _prep_inputs(x, Wq, bq, Wk, bk, Wv, bv):
    import ml_dtypes

    x = np.asarray(x, np.float32)
    Wq = np.asarray(Wq, np.float32)
    Wk = np.asarray(Wk, np.float32)
    Wv = np.asarray(Wv, np.float32)
    bq = np.asarray(bq, np.float32).ravel()
    bk = np.asarray(bk, np.float32).ravel()
    bv = np.asarray(bv, np.float32).ravel()

    def wprep(w, m):  # [768,m] -> [128, DT*m]: (p, dt*m+h) = w[dt*128+p, h]
        return np.ascontiguousarray(
            w.reshape(DT, 128, m).transpose(1, 0, 2).reshape(128, DT * m)
        ).astype(ml_dtypes.bfloat16)

    common = {
        "wA": wprep(np.concatenate([Wv, Wq], axis=1), 128),
        "wB": wprep(Wk, H),
        "bA": np.ascontiguousarray(np.concatenate([bv, bq]).reshape(128, 1)),
        "bB": np.ascontiguousarray(
            np.concatenate([np.zeros(H, np.float32), bk]).reshape(128, 1)
        ),
    }
    return x, common


def _xprep(xb):
    """[S, D] f32 -> [128, NQ, 2, DT*256] bf16:
    (p, c, h, dt*256+j) = x[c*512 + h*256 + j, dt*128 + p]"""
    import ml_dtypes

    t = xb.reshape(NQ, 2, 256, DT, 128).transpose(4, 0, 1, 3, 2)
    return np.ascontiguousarray(t.reshape(128, NQ, 2, DT * 256)).astype(
        ml_dtypes.bfloat16
    )


def _unshard_out(o):
    """[65, NQ*512] out^T with denominator row -> [S, H]"""
    o = np.asarray(o, np.float32)
    return (o[:H, :] / o[H : H + 1, :]).T


def _in_maps(x, common):
    return [{"xp": _xprep(x[b]), **common} for b in range(B)]


def kernel(x, Wq, bq, Wk, bk, Wv, bv, **_):
    from concourse.bass_utils import run_bass_kernel_spmd

    nc = _get_nc()
    x, common = _prep_inputs(x, Wq, bq, Wk, bk, Wv, bv)
    res = run_bass_kernel_spmd(nc, _in_maps(x, common), core_ids=list(range(B)))
    return np.stack([_unshard_out(res.results[b]["out"]) for b in range(B)])


# revision 59
# speedup vs baseline: 1.0586x; 1.0065x over previous
"""Single-head attention on 8 Trainium2 NeuronCores, batch-sharded.

Per core (one batch element b). Host-side layouts make every DMA a large
contiguous read (3KB/partition halves of x chunks).

Projections (bf16, chunk order 0..3, two fused groups):
  A [Wv|Wq] -> vq tile: rows 0-63 v^T, rows 64-127 q^T  (one DVE add)
  B [Wk]    -> kT tile rows 64-127 (M=64 matmul, tile_position (0,64));
               kT rows 0-63 are memset to zero once.
Scores then run the K=128 contraction directly on these tiles: the zero
rows of kT annihilate the v rows of vq, so NO zero-padded copies of q are
needed anywhere:  scores^T[kt] = kT[:,kt-cols]^T @ vq[:,q-cols].

Scores: 16 serial matmuls [K=128, M=128, N=512] per q-chunk at the warm
issue rate (~216ns); measured row-tiling of K=64 pairs gives NO speedup
(concurrent row-tiles serialize on the moving-operand SBUF port), so the
simple padded form wins (it needs 6144 fewer projection rows).
q-chunk-0 scores for k-tiles 4c..4c+3 are emitted right after chunk c's
projections, so the exp stream starts ~3us into the projection phase.

exp: split across TWO engines writing bf16 P^T tiles from [128,1536]
psum score tiles. ACT handles most groups (exact exp, scale=1/8 folded
in). DVE handles groups 0,3 of q-chunks 1-3 with a Schraudolph bit-trick:
bf16 bits of 2^y are linear in y, so
    bits = round(s_raw * (log2e/8 * 128) + 16248.5)
computed by ONE tensor_scalar (mult+add, f32 psum in, int16 out) IS
exp(s/8) to within ~2%; the int16 tile is bitcast to bf16 for the PV
matmul. This removes exp as the serial bottleneck (sim rel err of the
mix: 0.7e-2 < 2e-2 budget).

PV (bf16): per k-tile matmul, M=65 (V plus a ones row -> softmax
denominator row), accumulated over 16 k-tiles into a 1-bank psum.
PV(qc-1) interleaves with scores(qc); PV(3) trails its own exps by two
groups inside qc=3 to shorten the tail. outq psum banks alternate
between the "o" and "pj" tags (which projections also rotate through
during the projection phase).

V layout: per-chunk PE transposes of vq rows into v65 [128, kt, 80]
(pitch 160B) + DVE copy; a DMA-transpose variant exists but measured
slower (queue issue cost) — kept behind USE_DMA_TRANSPOSE.

Epilogue per q-chunk: DVE copy psum->SBUF, DMA out^T [65,512] f32; the
host does the divide-by-denominator and the final transpose (cheap).

PSUM: tag "sc" 2x3 banks (score tiles), tags "pj"+"o" 1 bank each
(projection psums, V-transpose staging, outq accumulators) = 8 banks.
"""

import numpy as np

USE_DMA_TRANSPOSE = False
VPITCH = 80  # v65 per-k-tile pitch: 160B — every multiple is 32B-aligned
             # (DMA-transpose dest requires 32B alignment)

B, S, D, H = 8, 2048, 768, 64
DT = D // 128          # 6 d-tiles
NQ = S // 512          # 4 q-chunks of 512
NK = S // 128          # 16 k-tiles of 128
SCALE = 1.0 / np.sqrt(H).item()
SCH_A = SCALE * np.log2(np.e).item() * 128.0   # Schraudolph slope
SCH_C = 16248.5                                 # Schraudolph offset (tuned)
GROUPS = ((0, 3), (3, 6), (6, 9), (9, 12), (12, 15), (15, 16))
DVE_GROUPS = {(qc, g) for qc in (1, 2, 3) for g in (0, 3)}

_cache = {}


def _build():
    import concourse.mybir as mybir
    import concourse.tile as tile
    from concourse import bacc
    from concourse.masks import make_identity

    f32 = mybir.dt.float32
    bf16 = mybir.dt.bfloat16
    i16 = mybir.dt.int16
    Exp = mybir.ActivationFunctionType.Exp
    Mult = mybir.AluOpType.mult
    Add = mybir.AluOpType.add

    nc = bacc.Bacc(None)
    xp_d = nc.dram_tensor("xp", [128, NQ, 2, DT * 256], bf16, kind="ExternalInput")
    wA_d = nc.dram_tensor("wA", [128, DT * 128], bf16, kind="ExternalInput")
    wB_d = nc.dram_tensor("wB", [128, DT * 64], bf16, kind="ExternalInput")
    bA_d = nc.dram_tensor("bA", [128, 1], f32, kind="ExternalInput")
    bB_d = nc.dram_tensor("bB", [128, 1], f32, kind="ExternalInput")
    out_d = nc.dram_tensor("out", [H + 1, NQ * 512], f32, kind="ExternalOutput")

    with tile.TileContext(nc) as tc:
        with (
            tc.tile_pool(name="big", bufs=1) as big,
            tc.tile_pool(name="small", bufs=1) as small,
            tc.tile_pool(name="pt", bufs=10) as ptp,
            tc.tile_pool(name="res", bufs=2) as resp,
            tc.tile_pool(name="ps", bufs=2, space="PSUM") as ps,
        ):
            # ---- PE warmup: matmuls on a DVE-memset tile during the DMA
            # fill push the HAM clock gate to K=8/8 before real work ----
            wsrc = small.tile([128, 128], bf16)
            nc.vector.memset(wsrc, 1.0)
            wps = ps.tile([128, 128], f32, tag="pj", name="warmps", bufs=1)
            for i in range(44):
                nc.tensor.matmul(wps, lhsT=wsrc, rhs=wsrc, start=True, stop=True)

            if not USE_DMA_TRANSPOSE:
                ident = small.tile([128, 128], f32)
                make_identity(nc, ident)
                identb = small.tile([128, 128], bf16)
                nc.gpsimd.tensor_copy(out=identb, in_=ident)

            # warm the ACT exp table during DMA fill
            zwarm = small.tile([128, 8], f32)
            nc.gpsimd.memset(zwarm, 0.0)
            wwarm = small.tile([128, 8], bf16)
            nc.scalar.activation(out=wwarm, in_=zwarm, func=Exp)

            # ---- DMAs, ordered so chunk-0 compute starts earliest ----
            wA = small.tile([128, DT, 128], bf16)
            wB = small.tile([128, DT, H], bf16)
            bA = small.tile([128, 1], f32)
            bB = small.tile([128, 1], f32)
            xT = big.tile([128, NQ, 2, DT * 256], bf16)

            nc.sync.dma_start(out=wA, in_=wA_d[:, :].rearrange("p (t h) -> p t h", t=DT))
            nc.sync.dma_start(out=bA, in_=bA_d[:, :])
            nc.sync.dma_start(out=xT[:, 0, 0, :], in_=xp_d[:, 0, 0, :])
            nc.sync.dma_start(out=xT[:, 0, 1, :], in_=xp_d[:, 0, 1, :])
            nc.sync.dma_start(out=wB, in_=wB_d[:, :].rearrange("p (t h) -> p t h", t=DT))
            nc.sync.dma_start(out=bB, in_=bB_d[:, :])
            for c in (1, 2, 3):
                for hh in (0, 1):
                    nc.sync.dma_start(out=xT[:, c, hh, :], in_=xp_d[:, c, hh, :])

            # ---- PE warmup: dummy matmuls during the DMA fill get the HAM
            # clock gate to K=8/8 (2.4 GHz) before the first real matmul ----
            # ---- persistent tensors ----
            vq = big.tile([128, S], bf16, tag="vq")   # v^T lo / q^T hi
            kT = big.tile([128, S], bf16, tag="kT")   # zeros lo / k^T hi
            nc.gpsimd.memset(kT[:H, :], 0.0)
            v65 = big.tile([128, NK, VPITCH], bf16, tag="v65")
            nc.gpsimd.memset(v65[:, :, H : H + 1], 1.0)

            # proj psums / vtrans staging / outq alternate two 1-bank tags
            alt = {"n": 0}

            def pj_tile(cols, nm, dtype=f32):
                tag = ("pj", "o")[alt["n"] % 2]
                alt["n"] += 1
                return ps.tile([128, cols], dtype, tag=tag, name=nm, bufs=1)

            pt_tiles = {}
            sc_state = {}
            outqs = [None] * NQ

            def emit_pv(qc, n):
                g = n // 3
                lo, _hi = GROUPS[g]
                pt = pt_tiles[(qc, g)]
                rhs_t = pt if pt.dtype == bf16 else pt.bitcast(bf16)
                slot = n - lo
                nc.tensor.matmul(
                    outqs[qc],
                    lhsT=v65[:, n, : H + 1],
                    rhs=rhs_t[:, slot * 512 : (slot + 1) * 512],
                    start=(n == 0),
                    stop=(n == NK - 1),
                )

            def emit_epilogue(qc, split=False):
                oT = resp.tile([H + 1, 512], f32, tag="oT", name=f"oT{qc}")
                if split:  # final epilogue: overlap the copy with the DMA
                    for hh in (0, 1):
                        nc.vector.tensor_copy(
                            out=oT[:, hh * 256 : (hh + 1) * 256],
                            in_=outqs[qc][:, hh * 256 : (hh + 1) * 256],
                        )
                        nc.sync.dma_start(
                            out=out_d[
                                :, qc * 512 + hh * 256 : qc * 512 + (hh + 1) * 256
                            ],
                            in_=oT[:, hh * 256 : (hh + 1) * 256],
                        )
                else:
                    nc.vector.tensor_copy(out=oT, in_=outqs[qc])
                    nc.sync.dma_start(
                        out=out_d[:, qc * 512 : (qc + 1) * 512], in_=oT
                    )

            def emit_score(qc, n):
                """scores^T for k-tile n against q-chunk qc (K=128 padded)."""
                st = sc_state.setdefault(qc, [None] * len(GROUPS))
                g = n // 3
                lo, hi = GROUPS[g]
                if st[g] is None:
                    st[g] = ps.tile(
                        [128, (hi - lo) * 512], f32, tag="sc", name=f"sc{qc}_{g}"
                    )
                slot = n - lo
                nc.tensor.matmul(
                    st[g][:, slot * 512 : (slot + 1) * 512],
                    lhsT=kT[:, n * 128 : (n + 1) * 128],
                    rhs=vq[:, qc * 512 : (qc + 1) * 512],
                    start=True,
                    stop=True,
                )
                if n == hi - 1:  # group full -> exp
                    cols = (hi - lo) * 512
                    if (qc, g) in DVE_GROUPS:
                        pt = ptp.tile([128, cols], i16, tag="pT", name=f"pt{qc}_{g}")
                        nc.vector.tensor_scalar(
                            out=pt,
                            in0=st[g],
                            scalar1=SCH_A,
                            scalar2=SCH_C,
                            op0=Mult,
                            op1=Add,
                        )
                    else:
                        pt = ptp.tile([128, cols], bf16, tag="pT", name=f"pt{qc}_{g}")
                        nc.scalar.activation(out=pt, in_=st[g], func=Exp, scale=SCALE)
                    pt_tiles[(qc, g)] = pt

            # ---- projection phase ----
            # Dependent work for chunk c (V transposes, q-chunk-0 scores)
            # is emitted AFTER chunk c+1's projection matmuls, so the DVE
            # bias-adds have a full matmul group's time to complete.
            def chunk_tail(c):
                for j in range(4):
                    kt = 4 * c + j
                    if USE_DMA_TRANSPOSE:
                        nc.sync.dma_start(
                            out=v65[:, kt, :H],
                            in_=vq[:H, kt * 128 : (kt + 1) * 128],
                            transpose=True,
                        )
                    else:
                        tp = pj_tile(128, f"vtr{kt}", dtype=bf16)
                        nc.tensor.transpose(
                            tp[:, :H],
                            vq[:H, kt * 128 : (kt + 1) * 128],
                            identb[:H, :H],
                        )
                        nc.vector.tensor_copy(out=v65[:, kt, :H], in_=tp[:, :H])
                for j in range(4):
                    emit_score(0, 4 * c + j)

            for c in range(4):
                cc = slice(c * 512, (c + 1) * 512)

                def proj(w, nm, mlo, mhi, c0=c):
                    p = pj_tile(512, f"ps{nm}{c0}")
                    for hh in (0, 1):
                        for dt in range(DT):
                            nc.tensor.matmul(
                                p[mlo:mhi, hh * 256 : (hh + 1) * 256],
                                lhsT=w[:, dt, :],
                                rhs=xT[:, c0, hh, dt * 256 : (dt + 1) * 256],
                                start=(dt == 0),
                                stop=(dt == DT - 1),
                            )
                    return p

                psA = proj(wA, "A", 0, 128)
                nc.vector.tensor_scalar_add(out=vq[:, cc], in0=psA, scalar1=bA)
                psB = proj(wB, "B", H, 128)
                nc.vector.tensor_scalar_add(
                    out=kT[H:, cc], in0=psB[H:, :], scalar1=bB[H:, :]
                )
                if c >= 1:
                    chunk_tail(c - 1)
            chunk_tail(3)

            # ---- steady phase ----
            pv_cursor = [0] * NQ

            def pump_pv(qc, limit_n):
                while pv_cursor[qc] < min(limit_n, NK):
                    emit_pv(qc, pv_cursor[qc])
                    pv_cursor[qc] += 1

            outqs[0] = ps.tile([H + 1, 512], f32, tag="o", name="outq0", bufs=1)
            for qc in range(1, NQ):
                for n4 in range(0, NK, 4):
                    for n in range(n4, n4 + 4):
                        emit_score(qc, n)
                    pump_pv(qc - 1, n4 + 4)
                    if qc == NQ - 1:
                        ge = sum(1 for g in range(6) if (qc, g) in pt_tiles)
                        if ge >= 3:
                            if outqs[qc] is None:
                                outqs[qc] = ps.tile(
                                    [H + 1, 512],
                                    f32,
                                    tag=("o", "pj")[qc % 2],
                                    name=f"outq{qc}",
                                    bufs=1,
                                )
                            pump_pv(qc, 3 * (ge - 2))
                emit_epilogue(qc - 1)
                if qc < NQ - 1:
                    outqs[qc] = ps.tile(
                        [H + 1, 512],
                        f32,
                        tag=("o", "pj")[qc % 2],
                        name=f"outq{qc}",
                        bufs=1,
                    )
            pump_pv(NQ - 1, NK)
            emit_epilogue(NQ - 1, split=True)

    nc.compile()
    return nc


def _get_nc():
    if "nc" not in _cache:
        _cache["nc"] = _build()
    return _cache["nc"]


def _prep_inputs(x, Wq, bq, Wk, bk, Wv, bv):
    import ml_dtypes

    x = np.asarray(x, np.float32)
    Wq = np.asarray(Wq, np.float32)
    Wk = np.asarray(Wk, np.float32)
    Wv = np.asarray(Wv, np.float32)
    bq = np.asarray(bq, np.float32).ravel()
    bk = np.asarray(bk, np.float32).ravel()
    bv = np.asarray(bv, np.float32).ravel()

    def wprep(w, m):  # [768,m] -> [128, DT*m]: (p, dt*m+h) = w[dt*128+p, h]
        return np.ascontiguousarray(
            w.reshape(DT, 128, m).transpose(1, 0, 2).reshape(128, DT * m)
        ).astype(ml_dtypes.bfloat16)

    common = {
        "wA": wprep(np.concatenate([Wv, Wq], axis=1), 128),
        "wB": wprep(Wk, H),
        "bA": np.ascontiguousarray(np.concatenate([bv, bq]).reshape(128, 1)),
        "bB": np.ascontiguousarray(
            np.concatenate([np.zeros(H, np.float32), bk]).reshape(128, 1)
        ),
    }
    return x, common


def _xprep(xb):
    """[S, D] f32 -> [128, NQ, 2, DT*256] bf16:
    (p, c, h, dt*256+j) = x[c*512 + h*256 + j, dt*128 + p]"""
    import ml_dtypes

    t = xb.reshape(NQ, 2, 256, DT, 128).transpose(4, 0, 1, 3, 2)
    return np.ascontiguousarray(t.reshape(128, NQ, 2, DT * 256)).astype(
        ml_dtypes.bfloat16
    )


def _unshard_out(o):
    """[65, NQ*512] out^T with denominator row -> [S, H]"""
    o = np.asarray(o, np.float32)
    return (o[:H, :] / o[H : H + 1, :]).T


def _in_maps(x, common):
    return [{"xp": _xprep(x[b]), **common} for b in range(B)]


def kernel(x, Wq, bq, Wk, bk, Wv, bv, **_):
    from concourse.bass_utils import run_bass_kernel_spmd

    nc = _get_nc()
    x, common = _prep_inputs(x, Wq, bq, Wk, bk, Wv, bv)
    res = run_bass_kernel_spmd(nc, _in_maps(x, common), core_ids=list(range(B)))
    return np.stack([_unshard_out(res.results[b]["out"]) for b in range(B)])
